# revision 15
# baseline (speedup 1.0000x reference)
"""Trainium2 Bass kernel for GQA attention layer (RoPE + causal + GQA 32q/8kv).

Self-contained: hardcodes shapes from the problem spec.
  hidden_states [2, 2048, 4096] f32, positions [2, 2048] i32,
  Wq [4096, 4096], Wk [1024, 4096], Wv [1024, 4096], Wo [4096, 4096]  (all f32)
Sharding: tensor-parallel over heads across 8 cores. Core c gets kv head c and
q heads 4c..4c+3. Each core computes its partial Wo output; host sums partials.

v3 changes over v2 (766us baseline):
- host-prearranged weight layouts so every weight DMA moves 8KB-contiguous
  runs per partition (vs 256B), killing the 13.6us startup stall
- x loaded as quarter-chunk tiles (bufs=7) with prefetch-ahead emission:
  full double-buffering so chunk-boundary PE stalls disappear
- queue separation: sync queue = x only, scalar queue = weights/cos/sin,
  gpsimd queue = rope swaps + output stores (no head-of-line blocking of
  x prefetches behind output DMAs)
- V transposes emitted after the first Q-head projection so the DVE has
  slack to produce vt before the PE needs it
- per-si4 output DMAs (finer tail, smaller opool)
- PE warm-up matmuls during the initial DMA wait (HAM at 2.4GHz from MM #1)
"""

import math
import os
import sys
import types

import numpy as np
import ml_dtypes

BF16NP = ml_dtypes.bfloat16

# ---- problem constants (hardcoded per spec) ----
P = 128
B = 2
S = 2048            # tokens per batch
HID = 4096
NH, NKV, HD = 32, 8, 128
NCORES = 8
HPC = NH // NCORES  # q heads per core (4)
T = B * S
SCALE = 1.0 / math.sqrt(HD)
ROPE_BASE = 10000.0

LAST = {}           # exec_time_ns etc from the most recent run


def _install_ntff_hook():
    """Register the axon NTFF profiling hook (image's antenv lacks axon_hooks)."""
    if "antenv.axon_hooks" in sys.modules:
        return
    try:
        import antenv
        mod = types.ModuleType("antenv.axon_hooks")
        _box = [None]
        mod.set_axon_ntff_profile_hook = lambda h: _box.__setitem__(0, h)
        mod.get_axon_ntff_profile_hook = lambda: _box[0]
        sys.modules["antenv.axon_hooks"] = mod
        antenv.axon_hooks = mod
        from trn_agent_boot.trn_boot import _ntff_profile_via_ctypes
        mod.set_axon_ntff_profile_hook(
            _ntff_profile_via_ctypes("/opt/axon/libaxon_pjrt.so")
        )
    except Exception:
        pass


def build_graph(S_=S, HID_=HID, CH=512, QC=512):
    import concourse.bacc as bacc
    import concourse.bass_isa as bass_isa
    import concourse.mybir as mybir
    import concourse.tile as tile
    from contextlib import ExitStack

    BF = mybir.dt.bfloat16
    F32 = mybir.dt.float32
    Exp = mybir.ActivationFunctionType.Exp

    NKK = HID_ // P          # contraction tiles over hidden (32)
    NCH = S_ // CH           # proj chunks per batch (4)
    NQC = S_ // QC           # attention q chunks per batch (4)
    NST = QC // P            # q subtiles per chunk (4)
    NKT = S_ // P            # k tiles per batch (16)
    NVS = CH // P            # v row-subtiles per proj chunk (4)
    HOC = HID_ // 512        # output column chunks (8)
    OCW = 512                # output chunk width
    NHG = HPC // 2           # head-pair passes per q chunk (2)
    NQ = 8                   # x slice tiles per chunk
    KPQ = NKK // NQ          # k-tiles per slice (4)

    nc = bacc.Bacc(None)
    xT_h = nc.declare_dram_parameter("xT", [HID_, B * S_], BF, isOutput=False)
    # prearranged weights: partition-major, contiguous per-partition slabs
    wq_h = nc.declare_dram_parameter("wqp", [P, HPC, NKK, HD], BF, isOutput=False)
    wk_h = nc.declare_dram_parameter("wkp", [P, NKK, HD], BF, isOutput=False)
    wv_h = nc.declare_dram_parameter("wvp", [P, NKK, HD], BF, isOutput=False)
    wo_h = nc.declare_dram_parameter("wop", [P, HPC, HID_], BF, isOutput=False)
    cos_h = nc.declare_dram_parameter("cos2", [P, S_], BF, isOutput=False)
    sin_h = nc.declare_dram_parameter("sin2", [P, S_], BF, isOutput=False)
    tri_h = nc.declare_dram_parameter("tri", [P, P], BF, isOutput=False)
    iden_h = nc.declare_dram_parameter("iden", [P, P], BF, isOutput=False)
    out_h = nc.declare_dram_parameter("out", [B * S_, HID_], BF, isOutput=True)

    xT_r = xT_h[:, :].rearrange("(ko ki) s -> ki ko s", ki=P)

    with tile.TileContext(nc) as tc, ExitStack() as ctx:
        wpool = ctx.enter_context(tc.tile_pool(name="wpool", bufs=1))
        qpool = ctx.enter_context(tc.tile_pool(name="qpool", bufs=1))
        kpool = ctx.enter_context(tc.tile_pool(name="kpool", bufs=1))
        vpool = ctx.enter_context(tc.tile_pool(name="vpool", bufs=1))
        xpool = ctx.enter_context(tc.tile_pool(name="xpool", bufs=13))
        cspool = ctx.enter_context(tc.tile_pool(name="cspool", bufs=1))
        rpool = ctx.enter_context(tc.tile_pool(name="rpool", bufs=2))
        vtpool = ctx.enter_context(tc.tile_pool(name="vtpool", bufs=2))
        pbpool = ctx.enter_context(tc.tile_pool(name="pbpool", bufs=10))
        dipool = ctx.enter_context(tc.tile_pool(name="dipool", bufs=2))
        ctpool = ctx.enter_context(tc.tile_pool(name="ctpool", bufs=2))
        opool = ctx.enter_context(tc.tile_pool(name="opool", bufs=3))

        psS = ctx.enter_context(tc.tile_pool(name="psS", bufs=4, space="PSUM"))
        psC = ctx.enter_context(tc.tile_pool(name="psC", bufs=2, space="PSUM"))
        psX = ctx.enter_context(tc.tile_pool(name="psX", bufs=2, space="PSUM"))

        # --- PE warm-up: dummy matmuls on a memset tile while DMAs stream in.
        # The HAM clock gate needs ~3.4us of PE busy to go 1.2->2.4GHz; these
        # run during the startup DMA wait so real matmuls start warm.
        ones_sb = wpool.tile([P, P], BF)
        nc.vector.memset(ones_sb, 1.0)
        for wi in range(32):
            pw = psS.tile([P, P], F32, name="pwarm", tag="s")
            nc.tensor.matmul(pw, lhsT=ones_sb, rhs=ones_sb, start=True, stop=True)

        # --- persistent weights / tables (scalar queue, prearranged layouts) ---
        wv_sb0 = None  # placeholder, reordered below
        wk_sb = wpool.tile([P, NKK, HD], BF)
        cos_sb = cspool.tile([P, S_], BF)
        sin_sb = cspool.tile([P, S_], BF)
        wv_sb = wpool.tile([P, NKK, HD], BF)
        nc.scalar.dma_start(out=wv_sb, in_=wv_h[:, :, :])
        nc.scalar.dma_start(out=wk_sb, in_=wk_h[:, :, :])
        nc.scalar.dma_start(out=cos_sb, in_=cos_h[:, :])
        nc.scalar.dma_start(out=sin_sb, in_=sin_h[:, :])
        wq_g = []
        for g in range(HPC):
            wgt = wpool.tile([P, NKK, HD], BF, name=f"wq{g}", tag=f"wq{g}")
            nc.scalar.dma_start(out=wgt, in_=wq_h[:, g, :, :])
            wq_g.append(wgt)
        iden_sb = wpool.tile([P, P], BF)
        nc.scalar.dma_start(out=iden_sb, in_=iden_h[:, :])
        tri_sb = wpool.tile([P, P], BF)
        nc.scalar.dma_start(out=tri_sb, in_=tri_h[:, :])
        wo_sb = wpool.tile([P, HPC, HID_], BF)
        for wi in range(4):
            lo, hi = wi * HID_ // 4, (wi + 1) * HID_ // 4
            nc.scalar.dma_start(out=wo_sb[:, :, lo:hi], in_=wo_h[:, :, lo:hi])

        # Wo-step interleave queue: closures that emit a bit of the previous
        # chunk's Wo GEMM (4 matmuls + a psum drain copy + out dma).
        pending = []

        def drain_wo(n):
            for _ in range(min(n, len(pending))):
                pending.pop(0)()

        def make_wo_steps(b, qc, ct):
            """Build the list of Wo steps for q-chunk qc of batch b."""
            steps = []

            def step(hc, si4):
                def run():
                    po = psX.tile([P, OCW], F32, name="po", tag="px")
                    for ot in range(HPC):
                        nc.tensor.matmul(
                            po,
                            lhsT=ct[:, ot, si4 * P:(si4 + 1) * P],
                            rhs=wo_sb[:, ot, hc * OCW:(hc + 1) * OCW],
                            start=(ot == 0), stop=(ot == HPC - 1),
                        )
                    ob = opool.tile([P, OCW], BF, name="ob", tag="ob")
                    nc.vector.tensor_copy(out=ob, in_=po)
                    r0 = b * S_ + qc * QC + si4 * P
                    nc.sync.dma_start(
                        out=out_h[r0:r0 + P, hc * OCW:(hc + 1) * OCW], in_=ob
                    )
                return run

            for hc in range(HOC):
                for si4 in range(NST):
                    steps.append(step(hc, si4))
            return steps

        def rope(ps, dst, cs, sn):
            """Neox RoPE on [128 d, n] tile: rows 0:64 = first half of head dim."""
            qf = rpool.tile([P, CH], BF, tag="qf")
            nc.vector.tensor_copy(out=qf, in_=ps)
            qs = rpool.tile([P, CH], BF, tag="qs")
            nc.gpsimd.dma_start(out=qs[0:64, :], in_=qf[64:128, :])
            nc.gpsimd.dma_start(out=qs[64:128, :], in_=qf[0:64, :])
            nc.vector.tensor_mul(out=qf, in0=qf, in1=cs)
            nc.vector.tensor_mul(out=qs, in0=qs, in1=sn)
            nc.vector.tensor_add(out=dst, in0=qf, in1=qs)

        def load_x_quarters(b, t, engines):
            """Emit the 4 quarter-tile DMAs for chunk t of batch b."""
            c0 = b * S_ + t * CH
            tiles = []
            for q in range(NQ):
                xq = xpool.tile([P, KPQ, CH], BF, tag="x")
                eng = engines[q % len(engines)]
                eng.dma_start(out=xq, in_=xT_r[:, q * KPQ:(q + 1) * KPQ, c0:c0 + CH])
                tiles.append(xq)
            return tiles

        # first chunk: sync queue (hardware DGE; gpsimd's queue is software-DGE
        # and far too slow for 1KB-run transfers)
        cur_x = load_x_quarters(0, 0, [nc.sync])

        for b in range(B):
            # ---------- phase P: projections + RoPE ----------
            qT = qpool.tile([P, HPC, S_], BF)
            kT = kpool.tile([P, S_], BF)
            v = vpool.tile([P, NKT, P], BF)
            for t in range(NCH):
                # prefetch next chunk's x (sync queue only)
                if not (b == B - 1 and t == NCH - 1):
                    nb, nt = (b, t + 1) if t + 1 < NCH else (b + 1, 0)
                    nxt_x = load_x_quarters(nb, nt, [nc.sync])
                else:
                    nxt_x = None
                cs = cos_sb[:, t * CH:(t + 1) * CH]
                sn = sin_sb[:, t * CH:(t + 1) * CH]

                def xt(kk):
                    return cur_x[kk // KPQ][:, kk % KPQ, :]

                # V projection first: it has no rope, so the kernel can
                # start on just wv + the first x slices. Projection
                # accumulators draw from psS (4 bufs, idle during phase P) so
                # the 2-buf psX ring is left to the interleaved Wo drains.
                pv = psS.tile([P, CH], F32, tag="s")
                for kk in range(NKK):
                    nc.tensor.matmul(
                        pv, lhsT=wv_sb[:, kk, :], rhs=xt(kk),
                        start=(kk == 0), stop=(kk == NKK - 1),
                    )
                vt = vtpool.tile([P, CH], BF, tag="vt")
                nc.vector.tensor_copy(out=vt, in_=pv)
                drain_wo(2)
                ps = psS.tile([P, CH], F32, tag="s")
                for kk in range(NKK):
                    nc.tensor.matmul(
                        ps, lhsT=wk_sb[:, kk, :], rhs=xt(kk),
                        start=(kk == 0), stop=(kk == NKK - 1),
                    )
                rope(ps, kT[:, t * CH:t * CH + CH], cs, sn)
                drain_wo(2)
                for g in range(HPC):
                    ps = psS.tile([P, CH], F32, tag="s")
                    for kk in range(NKK):
                        nc.tensor.matmul(
                            ps,
                            lhsT=wq_g[g][:, kk, :],
                            rhs=xt(kk),
                            start=(kk == 0), stop=(kk == NKK - 1),
                        )
                    rope(ps, qT[:, g, t * CH:t * CH + CH], cs, sn)
                    drain_wo(2)
                    if g == 0:
                        # PE-transposes of v, deferred so the DVE has slack
                        # to produce vt before the PE needs it
                        for ss in range(NVS):
                            pq = psS.tile([P, P], BF, tag="s")
                            nc.tensor.transpose(
                                pq, vt[:, ss * P:(ss + 1) * P], iden_sb
                            )
                            nc.scalar.copy(out=v[:, t * NVS + ss, :], in_=pq)
                        drain_wo(2)
                cur_x = nxt_x

            # ---------- phase A: attention ----------
            for qc in range(NQC):
                nkt = (qc + 1) * NST
                ct = ctpool.tile([P, HPC, QC], BF, name="ct", tag="ct")
                # distribute leftover Wo work evenly over this chunk's iters
                niter = NHG * (nkt + 3)
                quota = len(pending) / max(niter, 1)
                acc = [0.0]

                def drain_quota():
                    acc[0] += quota
                    k = int(acc[0])
                    if k:
                        acc[0] -= k
                        drain_wo(k)

                for hg in range(NHG):
                    h0 = hg * 2
                    pcs = {}
                    pdb = {}
                    pbs = {}
                    ph1 = {}
                    for kt in range(nkt + 3):
                        if kt < nkt:
                            d = kt - qc * NST  # diag subtile index if >= 0
                            lo = d * P if d > 0 else 0
                            pss = {}
                            for hh in range(2):
                                h = h0 + hh
                                pt = psS.tile([P, QC], F32, name="pss", tag="s")
                                nc.tensor.matmul(
                                    pt[:, lo:QC],
                                    lhsT=kT[:, kt * P:(kt + 1) * P],
                                    rhs=qT[:, h, qc * QC + lo:(qc + 1) * QC],
                                    start=True, stop=True,
                                )
                                pss[hh] = pt
                            for hh in range(2):
                                pb = pbpool.tile([P, QC], BF, name="pb", tag="pb")
                                nc.scalar.activation(
                                    out=pb[:, lo:QC], in_=pss[hh][:, lo:QC],
                                    func=Exp, scale=SCALE,
                                )
                                if d >= 0:
                                    # zero strictly-upper part of the diagonal
                                    # 128-subtile post-exp (keep k row <= q col)
                                    nc.vector.tensor_mul(
                                        out=pb[:, d * P:(d + 1) * P],
                                        in0=pb[:, d * P:(d + 1) * P],
                                        in1=tri_sb,
                                    )
                                if lo:
                                    nc.gpsimd.memset(pb[:, 0:lo], 0.0)
                                pbs[(kt, hh)] = pb
                        if kt > 2:
                            kp = kt - 3
                            first, last = (kp == 0), (kp == nkt - 1)
                            dp = kp - qc * NST
                            lop = dp * P if dp > 0 else 0
                            for hh in range(2):
                                pb = pbs[(kp, hh)]
                                if first:
                                    pcs[hh] = psC.tile(
                                        [P, QC], F32, name="pctx", tag="ctx"
                                    )
                                nc.tensor.matmul(
                                    pcs[hh][:, lop:QC],
                                    lhsT=v[:, kp, :],
                                    rhs=pb[:, lop:QC],
                                    start=first, stop=last,
                                )
                                # denominator: sum groups of 4 kt on vector,
                                # one all-ones matmul per group (broadcast D)
                                if kp % 4 == 1:
                                    s2 = pbpool.tile(
                                        [P, QC], BF, name="s2", tag="s2", bufs=3
                                    )
                                    nc.vector.tensor_add(
                                        out=s2, in0=pbs.pop((kp - 1, hh)), in1=pb
                                    )
                                    del pbs[(kp, hh)]
                                    ph1[hh] = s2
                                elif kp % 4 == 3:
                                    s2 = pbpool.tile(
                                        [P, QC], BF, name="s2b", tag="s2b", bufs=2
                                    )
                                    nc.vector.tensor_add(
                                        out=s2, in0=pbs.pop((kp - 1, hh)), in1=pb
                                    )
                                    del pbs[(kp, hh)]
                                    s4 = pbpool.tile(
                                        [P, QC], BF, name="s4", tag="s4", bufs=3
                                    )
                                    nc.vector.tensor_add(
                                        out=s4, in0=ph1.pop(hh), in1=s2
                                    )
                                    if hh in pdb:
                                        nc.vector.tensor_add(
                                            out=s4, in0=pdb[hh], in1=s4
                                        )
                                    pdb[hh] = s4
                        drain_quota()
                    # normalize this head pair: ctxT = pcs * (1/denominator).
                    # Cross-partition sum on gpsimd (frees the PE of the
                    # all-ones broadcast matmul and the psX pd allocations).
                    for hh in range(2):
                        dsum = dipool.tile([P, QC], F32, name="dsum", tag="ds")
                        nc.gpsimd.partition_all_reduce(
                            dsum[:, :], pdb[hh][:, :],
                            channels=P, reduce_op=bass_isa.ReduceOp.add,
                        )
                        dinv = dipool.tile([P, QC], F32, name="dinv", tag="di")
                        nc.vector.reciprocal_approx_fast(out=dinv, in_=dsum)
                        nc.vector.tensor_mul(
                            out=ct[:, h0 + hh, :], in0=pcs[hh], in1=dinv
                        )
                drain_wo(len(pending))  # anything left from previous chunk
                pending = make_wo_steps(b, qc, ct)

        drain_wo(len(pending))

    nc.compile()
    return nc


_CACHE = {}


def _get_graph():
    if "nc" not in _CACHE:
        _CACHE["nc"] = build_graph()
    return _CACHE["nc"]


def _host_prep(hidden_states, positions, Wq, Wk, Wv, Wo):
    """Transpose/cast/prearrange inputs per core. Returns list of 8 input dicts.

    Weight tensors are prearranged so that each DMA moves long contiguous
    per-partition runs:
      wkp/wvp[ki, ko, d]   = W.T[ko*128+ki, d]          (8KB runs)
      wqp[ki, g, ko, d]    = Wq.T[ko*128+ki, g*128+d]   (8KB runs per head)
      wop[oi, oo, h]       = Wo.T[oo*128+oi, h]         (2KB runs per slice)
    """
    x2 = np.ascontiguousarray(hidden_states.reshape(T, HID).T).astype(BF16NP)

    pos = positions.astype(np.float32)                      # [B, S]
    assert np.array_equal(positions[0], positions[1]), (
        "kernel assumes identical positions across batch"
    )
    half = HD // 2
    inv_freq = 1.0 / (ROPE_BASE ** (np.arange(half, dtype=np.float32) / half))
    ang = pos[0][:, None] * inv_freq[None, :]               # [S, 64]
    cosT = np.cos(ang).T                                    # [64, S]
    sinT = np.sin(ang).T
    cos2 = np.concatenate([cosT, cosT], axis=0).astype(BF16NP)    # [128, S]
    sin2 = np.concatenate([-sinT, sinT], axis=0).astype(BF16NP)

    r = np.arange(P)
    tri = (r[:, None] <= r[None, :]).astype(np.float32).astype(BF16NP)
    iden = np.eye(P, dtype=np.float32).astype(BF16NP)

    NKK = HID // P

    def prearrange_kd(wT):            # [HID, d] -> [ki, ko, d]
        d = wT.shape[1]
        return np.ascontiguousarray(
            wT.reshape(NKK, P, d).transpose(1, 0, 2)
        )

    in_maps = []
    for c in range(NCORES):
        qs = slice(c * HPC * HD, (c + 1) * HPC * HD)
        ks = slice(c * HD, (c + 1) * HD)
        wqT = Wq[qs, :].T.astype(BF16NP)                     # [HID, 512]
        wkT = Wk[ks, :].T.astype(BF16NP)                     # [HID, 128]
        wvT = Wv[ks, :].T.astype(BF16NP)
        woT = Wo[:, qs].T.astype(BF16NP)                     # [512, HID]
        wqp = np.ascontiguousarray(
            wqT.reshape(NKK, P, HPC, HD).transpose(1, 2, 0, 3)
        )                                                    # [ki, g, ko, d]
        wop = np.ascontiguousarray(
            woT.reshape(HPC, P, HID).transpose(1, 0, 2)
        )                                                    # [oi, oo, h]
        in_maps.append({
            "xT": x2,
            "wqp": wqp,
            "wkp": prearrange_kd(wkT),
            "wvp": prearrange_kd(wvT),
            "wop": wop,
            "cos2": cos2,
            "sin2": sin2,
            "tri": tri,
            "iden": iden,
        })
    return in_maps


def kernel(hidden_states, positions, Wq, Wk, Wv, Wo):
    from concourse.bass_utils import run_bass_kernel_spmd

    trace = bool(os.environ.get("CLAUDE_KERNEL_TRACE"))
    if trace:
        _install_ntff_hook()

    nc = _get_graph()
    in_maps = _host_prep(
        np.asarray(hidden_states), np.asarray(positions),
        np.asarray(Wq), np.asarray(Wk), np.asarray(Wv), np.asarray(Wo),
    )
    res = run_bass_kernel_spmd(
        nc, in_maps, core_ids=list(range(NCORES)), trace=trace,
    )
    LAST["exec_time_ns"] = res.exec_time_ns
    LAST["profile_json"] = res.profile_json
    if res.instructions_and_trace is not None:
        LAST["trace_path"] = res.instructions_and_trace[1]

    acc = np.zeros((T, HID), np.float32)
    for c in range(NCORES):
        acc += res.results[c]["out"].astype(np.float32)
    return acc.reshape(B, S, HID)


# revision 16
# speedup vs baseline: 1.1068x; 1.1068x over previous
"""Trainium2 Bass kernel for GQA attention layer (RoPE + causal + GQA 32q/8kv).

Self-contained: hardcodes shapes from the problem spec.
  hidden_states [2, 2048, 4096] f32, positions [2, 2048] i32,
  Wq [4096, 4096], Wk [1024, 4096], Wv [1024, 4096], Wo [4096, 4096]  (all f32)
Sharding: tensor-parallel over heads across 8 cores. Core c gets kv head c and
q heads 4c..4c+3. Each core computes its partial Wo output; host sums partials.

v3 changes over v2 (766us baseline):
- host-prearranged weight layouts so every weight DMA moves 8KB-contiguous
  runs per partition (vs 256B), killing the 13.6us startup stall
- x loaded as quarter-chunk tiles (bufs=7) with prefetch-ahead emission:
  full double-buffering so chunk-boundary PE stalls disappear
- queue separation: sync queue = x only, scalar queue = weights/cos/sin,
  gpsimd queue = rope swaps + output stores (no head-of-line blocking of
  x prefetches behind output DMAs)
- V transposes emitted after the first Q-head projection so the DVE has
  slack to produce vt before the PE needs it
- per-si4 output DMAs (finer tail, smaller opool)
- PE warm-up matmuls during the initial DMA wait (HAM at 2.4GHz from MM #1)
"""

import math
import os
import sys
import types

import numpy as np
import ml_dtypes

BF16NP = ml_dtypes.bfloat16

# ---- problem constants (hardcoded per spec) ----
P = 128
B = 2
S = 2048            # tokens per batch
HID = 4096
NH, NKV, HD = 32, 8, 128
NCORES = 8
HPC = NH // NCORES  # q heads per core (4)
T = B * S
SCALE = 1.0 / math.sqrt(HD)
ROPE_BASE = 10000.0

LAST = {}           # exec_time_ns etc from the most recent run


def _install_ntff_hook():
    """Register the axon NTFF profiling hook (image's antenv lacks axon_hooks)."""
    if "antenv.axon_hooks" in sys.modules:
        return
    try:
        import antenv
        mod = types.ModuleType("antenv.axon_hooks")
        _box = [None]
        mod.set_axon_ntff_profile_hook = lambda h: _box.__setitem__(0, h)
        mod.get_axon_ntff_profile_hook = lambda: _box[0]
        sys.modules["antenv.axon_hooks"] = mod
        antenv.axon_hooks = mod
        from trn_agent_boot.trn_boot import _ntff_profile_via_ctypes
        mod.set_axon_ntff_profile_hook(
            _ntff_profile_via_ctypes("/opt/axon/libaxon_pjrt.so")
        )
    except Exception:
        pass


def build_graph(S_=S, HID_=HID, CH=512, QC=512):
    import concourse.bacc as bacc
    import concourse.bass_isa as bass_isa
    import concourse.mybir as mybir
    import concourse.tile as tile
    from contextlib import ExitStack

    BF = mybir.dt.bfloat16
    F32 = mybir.dt.float32
    Exp = mybir.ActivationFunctionType.Exp

    NKK = HID_ // P          # contraction tiles over hidden (32)
    NCH = S_ // CH           # proj chunks per batch (4)
    NQC = S_ // QC           # attention q chunks per batch (4)
    NST = QC // P            # q subtiles per chunk (4)
    NKT = S_ // P            # k tiles per batch (16)
    NVS = CH // P            # v row-subtiles per proj chunk (4)
    HOC = HID_ // 512        # output column chunks (8)
    OCW = 512                # output chunk width
    NHG = HPC // 2           # head-pair passes per q chunk (2)
    NQ = 8                   # x slice tiles per chunk
    KPQ = NKK // NQ          # k-tiles per slice (4)

    nc = bacc.Bacc(None)
    xT_h = nc.declare_dram_parameter("xT", [HID_, B * S_], BF, isOutput=False)
    # prearranged weights: partition-major, contiguous per-partition slabs
    wq_h = nc.declare_dram_parameter("wqp", [P, HPC, NKK, HD], BF, isOutput=False)
    wk_h = nc.declare_dram_parameter("wkp", [P, NKK, HD], BF, isOutput=False)
    wv_h = nc.declare_dram_parameter("wvp", [P, NKK, HD], BF, isOutput=False)
    wo_h = nc.declare_dram_parameter("wop", [P, HPC, HID_], BF, isOutput=False)
    cos_h = nc.declare_dram_parameter("cos2", [P, S_], BF, isOutput=False)
    sin_h = nc.declare_dram_parameter("sin2", [P, S_], BF, isOutput=False)
    tri_h = nc.declare_dram_parameter("tri", [P, P], BF, isOutput=False)
    iden_h = nc.declare_dram_parameter("iden", [P, P], BF, isOutput=False)
    out_h = nc.declare_dram_parameter("out", [B * S_, HID_], BF, isOutput=True)

    xT_r = xT_h[:, :].rearrange("(ko ki) s -> ki ko s", ki=P)

    with tile.TileContext(nc) as tc, ExitStack() as ctx:
        wpool = ctx.enter_context(tc.tile_pool(name="wpool", bufs=1))
        qpool = ctx.enter_context(tc.tile_pool(name="qpool", bufs=1))
        kpool = ctx.enter_context(tc.tile_pool(name="kpool", bufs=1))
        vpool = ctx.enter_context(tc.tile_pool(name="vpool", bufs=1))
        xpool = ctx.enter_context(tc.tile_pool(name="xpool", bufs=13))
        cspool = ctx.enter_context(tc.tile_pool(name="cspool", bufs=1))
        rpool = ctx.enter_context(tc.tile_pool(name="rpool", bufs=2))
        vtpool = ctx.enter_context(tc.tile_pool(name="vtpool", bufs=2))
        pbpool = ctx.enter_context(tc.tile_pool(name="pbpool", bufs=10))
        dipool = ctx.enter_context(tc.tile_pool(name="dipool", bufs=2))
        ctpool = ctx.enter_context(tc.tile_pool(name="ctpool", bufs=2))
        opool = ctx.enter_context(tc.tile_pool(name="opool", bufs=3))

        psS = ctx.enter_context(tc.tile_pool(name="psS", bufs=4, space="PSUM"))
        psC = ctx.enter_context(tc.tile_pool(name="psC", bufs=2, space="PSUM"))
        psX = ctx.enter_context(tc.tile_pool(name="psX", bufs=2, space="PSUM"))

        # --- PE warm-up: dummy matmuls on a memset tile while DMAs stream in.
        # The HAM clock gate needs ~3.4us of PE busy to go 1.2->2.4GHz; these
        # run during the startup DMA wait so real matmuls start warm.
        ones_sb = wpool.tile([P, P], BF)
        nc.vector.memset(ones_sb, 1.0)
        for wi in range(32):
            pw = psS.tile([P, P], F32, name="pwarm", tag="s")
            nc.tensor.matmul(pw, lhsT=ones_sb, rhs=ones_sb, start=True, stop=True)

        # --- persistent weights / tables (scalar queue, prearranged layouts) ---
        wv_sb0 = None  # placeholder, reordered below
        wk_sb = wpool.tile([P, NKK, HD], BF)
        cos_sb = cspool.tile([P, S_], BF)
        sin_sb = cspool.tile([P, S_], BF)
        wv_sb = wpool.tile([P, NKK, HD], BF)
        nc.scalar.dma_start(out=wv_sb, in_=wv_h[:, :, :])
        nc.scalar.dma_start(out=wk_sb, in_=wk_h[:, :, :])
        nc.scalar.dma_start(out=cos_sb, in_=cos_h[:, :])
        nc.scalar.dma_start(out=sin_sb, in_=sin_h[:, :])
        wq_g = []
        for g in range(HPC):
            wgt = wpool.tile([P, NKK, HD], BF, name=f"wq{g}", tag=f"wq{g}")
            nc.scalar.dma_start(out=wgt, in_=wq_h[:, g, :, :])
            wq_g.append(wgt)
        iden_sb = wpool.tile([P, P], BF)
        nc.scalar.dma_start(out=iden_sb, in_=iden_h[:, :])
        tri_sb = wpool.tile([P, P], BF)
        nc.scalar.dma_start(out=tri_sb, in_=tri_h[:, :])
        wo_sb = wpool.tile([P, HPC, HID_], BF)
        for wi in range(4):
            lo, hi = wi * HID_ // 4, (wi + 1) * HID_ // 4
            nc.scalar.dma_start(out=wo_sb[:, :, lo:hi], in_=wo_h[:, :, lo:hi])

        # Wo-step interleave queue: closures that emit a bit of the previous
        # chunk's Wo GEMM (4 matmuls + a psum drain copy + out dma).
        pending = []

        def drain_wo(n):
            for _ in range(min(n, len(pending))):
                pending.pop(0)()

        def make_wo_steps(b, qc, ct):
            """Build the list of Wo steps for q-chunk qc of batch b."""
            steps = []

            def step(hc, si4):
                def run():
                    po = psX.tile([P, OCW], F32, name="po", tag="px")
                    for ot in range(HPC):
                        nc.tensor.matmul(
                            po,
                            lhsT=ct[:, ot, si4 * P:(si4 + 1) * P],
                            rhs=wo_sb[:, ot, hc * OCW:(hc + 1) * OCW],
                            start=(ot == 0), stop=(ot == HPC - 1),
                        )
                    ob = opool.tile([P, OCW], BF, name="ob", tag="ob")
                    nc.vector.tensor_copy(out=ob, in_=po)
                    r0 = b * S_ + qc * QC + si4 * P
                    nc.sync.dma_start(
                        out=out_h[r0:r0 + P, hc * OCW:(hc + 1) * OCW], in_=ob
                    )
                return run

            for hc in range(HOC):
                for si4 in range(NST):
                    steps.append(step(hc, si4))
            return steps

        def rope(ps, dst, cs, sn):
            """Neox RoPE on [128 d, n] tile: rows 0:64 = first half of head dim."""
            qf = rpool.tile([P, CH], BF, tag="qf")
            nc.vector.tensor_copy(out=qf, in_=ps)
            qs = rpool.tile([P, CH], BF, tag="qs")
            nc.gpsimd.dma_start(out=qs[0:64, :], in_=qf[64:128, :])
            nc.gpsimd.dma_start(out=qs[64:128, :], in_=qf[0:64, :])
            nc.vector.tensor_mul(out=qf, in0=qf, in1=cs)
            nc.vector.tensor_mul(out=qs, in0=qs, in1=sn)
            nc.vector.tensor_add(out=dst, in0=qf, in1=qs)

        def load_x_quarters(b, t, engines):
            """Emit the 4 quarter-tile DMAs for chunk t of batch b."""
            c0 = b * S_ + t * CH
            tiles = []
            for q in range(NQ):
                xq = xpool.tile([P, KPQ, CH], BF, tag="x")
                eng = engines[q % len(engines)]
                eng.dma_start(out=xq, in_=xT_r[:, q * KPQ:(q + 1) * KPQ, c0:c0 + CH])
                tiles.append(xq)
            return tiles

        # first chunk: sync queue (hardware DGE; gpsimd's queue is software-DGE
        # and far too slow for 1KB-run transfers)
        cur_x = load_x_quarters(0, 0, [nc.sync])

        for b in range(B):
            # ---------- phase P: projections + RoPE ----------
            qT = qpool.tile([P, HPC, S_], BF)
            kT = kpool.tile([P, S_], BF)
            v = vpool.tile([P, NKT, P], BF)
            for t in range(NCH):
                # prefetch next chunk's x (sync queue only)
                if not (b == B - 1 and t == NCH - 1):
                    nb, nt = (b, t + 1) if t + 1 < NCH else (b + 1, 0)
                    nxt_x = load_x_quarters(nb, nt, [nc.sync])
                else:
                    nxt_x = None
                cs = cos_sb[:, t * CH:(t + 1) * CH]
                sn = sin_sb[:, t * CH:(t + 1) * CH]

                def xt(kk):
                    return cur_x[kk // KPQ][:, kk % KPQ, :]

                # V projection first: it has no rope, so the kernel can
                # start on just wv + the first x slices. Projection
                # accumulators draw from psS (4 bufs, idle during phase P) so
                # the 2-buf psX ring is left to the interleaved Wo drains.
                pv = psS.tile([P, CH], F32, tag="s")
                for kk in range(NKK):
                    nc.tensor.matmul(
                        pv, lhsT=wv_sb[:, kk, :], rhs=xt(kk),
                        start=(kk == 0), stop=(kk == NKK - 1),
                    )
                vt = vtpool.tile([P, CH], BF, tag="vt")
                nc.vector.tensor_copy(out=vt, in_=pv)
                drain_wo(2)
                ps = psS.tile([P, CH], F32, tag="s")
                for kk in range(NKK):
                    nc.tensor.matmul(
                        ps, lhsT=wk_sb[:, kk, :], rhs=xt(kk),
                        start=(kk == 0), stop=(kk == NKK - 1),
                    )
                rope(ps, kT[:, t * CH:t * CH + CH], cs, sn)
                drain_wo(2)
                for g in range(HPC):
                    ps = psS.tile([P, CH], F32, tag="s")
                    for kk in range(NKK):
                        nc.tensor.matmul(
                            ps,
                            lhsT=wq_g[g][:, kk, :],
                            rhs=xt(kk),
                            start=(kk == 0), stop=(kk == NKK - 1),
                        )
                    rope(ps, qT[:, g, t * CH:t * CH + CH], cs, sn)
                    drain_wo(2)
                    if g == 0:
                        # PE-transposes of v, deferred so the DVE has slack
                        # to produce vt before the PE needs it
                        for ss in range(NVS):
                            pq = psS.tile([P, P], BF, tag="s")
                            nc.tensor.transpose(
                                pq, vt[:, ss * P:(ss + 1) * P], iden_sb
                            )
                            nc.scalar.copy(out=v[:, t * NVS + ss, :], in_=pq)
                        drain_wo(2)
                cur_x = nxt_x

            # ---------- phase A: attention ----------
            for qc in range(NQC):
                nkt = (qc + 1) * NST
                ct = ctpool.tile([P, HPC, QC], BF, name="ct", tag="ct")
                # distribute leftover Wo work evenly over this chunk's iters
                niter = NHG * (nkt + 3)
                quota = len(pending) / max(niter, 1)
                acc = [0.0]

                def drain_quota():
                    acc[0] += quota
                    k = int(acc[0])
                    if k:
                        acc[0] -= k
                        drain_wo(k)

                for hg in range(NHG):
                    h0 = hg * 2
                    pcs = {}
                    pdb = {}
                    pbs = {}
                    ph1 = {}
                    for kt in range(nkt + 3):
                        if kt < nkt:
                            d = kt - qc * NST  # diag subtile index if >= 0
                            lo = d * P if d > 0 else 0
                            pss = {}
                            for hh in range(2):
                                h = h0 + hh
                                pt = psS.tile([P, QC], F32, name="pss", tag="s")
                                nc.tensor.matmul(
                                    pt[:, lo:QC],
                                    lhsT=kT[:, kt * P:(kt + 1) * P],
                                    rhs=qT[:, h, qc * QC + lo:(qc + 1) * QC],
                                    start=True, stop=True,
                                )
                                pss[hh] = pt
                            for hh in range(2):
                                pb = pbpool.tile([P, QC], BF, name="pb", tag="pb")
                                nc.scalar.activation(
                                    out=pb[:, lo:QC], in_=pss[hh][:, lo:QC],
                                    func=Exp, scale=SCALE,
                                )
                                if d >= 0:
                                    # zero strictly-upper part of the diagonal
                                    # 128-subtile post-exp (keep k row <= q col)
                                    nc.vector.tensor_mul(
                                        out=pb[:, d * P:(d + 1) * P],
                                        in0=pb[:, d * P:(d + 1) * P],
                                        in1=tri_sb,
                                    )
                                if lo:
                                    nc.gpsimd.memset(pb[:, 0:lo], 0.0)
                                pbs[(kt, hh)] = pb
                        if kt > 2:
                            kp = kt - 3
                            first, last = (kp == 0), (kp == nkt - 1)
                            dp = kp - qc * NST
                            lop = dp * P if dp > 0 else 0
                            for hh in range(2):
                                pb = pbs[(kp, hh)]
                                if first:
                                    pcs[hh] = psC.tile(
                                        [P, QC], F32, name="pctx", tag="ctx"
                                    )
                                nc.tensor.matmul(
                                    pcs[hh][:, lop:QC],
                                    lhsT=v[:, kp, :],
                                    rhs=pb[:, lop:QC],
                                    start=first, stop=last,
                                )
                                # denominator: sum groups of 4 kt on vector,
                                # one all-ones matmul per group (broadcast D)
                                if kp % 4 == 1:
                                    s2 = pbpool.tile(
                                        [P, QC], BF, name="s2", tag="s2", bufs=3
                                    )
                                    nc.vector.tensor_add(
                                        out=s2, in0=pbs.pop((kp - 1, hh)), in1=pb
                                    )
                                    del pbs[(kp, hh)]
                                    ph1[hh] = s2
                                elif kp % 4 == 3:
                                    s2 = pbpool.tile(
                                        [P, QC], BF, name="s2b", tag="s2b", bufs=2
                                    )
                                    nc.vector.tensor_add(
                                        out=s2, in0=pbs.pop((kp - 1, hh)), in1=pb
                                    )
                                    del pbs[(kp, hh)]
                                    s4 = pbpool.tile(
                                        [P, QC], BF, name="s4", tag="s4", bufs=3
                                    )
                                    nc.vector.tensor_add(
                                        out=s4, in0=ph1.pop(hh), in1=s2
                                    )
                                    if hh in pdb:
                                        nc.vector.tensor_add(
                                            out=s4, in0=pdb[hh], in1=s4
                                        )
                                    pdb[hh] = s4
                        drain_quota()
                    # normalize this head pair: ctxT = pcs * (1/denominator)
                    for hh in range(2):
                        pd = psX.tile([P, QC], F32, name="pd", tag="px")
                        nc.tensor.matmul(
                            pd, lhsT=ones_sb, rhs=pdb[hh],
                            start=True, stop=True,
                        )
                        dinv = dipool.tile([P, QC], F32, name="dinv", tag="di")
                        nc.vector.reciprocal_approx_fast(out=dinv, in_=pd)
                        nc.vector.tensor_mul(
                            out=ct[:, h0 + hh, :], in0=pcs[hh], in1=dinv
                        )
                drain_wo(len(pending))  # anything left from previous chunk
                pending = make_wo_steps(b, qc, ct)

        drain_wo(len(pending))

    nc.compile()
    return nc


_CACHE = {}


def _get_graph():
    if "nc" not in _CACHE:
        _CACHE["nc"] = build_graph()
    return _CACHE["nc"]


def _host_prep(hidden_states, positions, Wq, Wk, Wv, Wo):
    """Transpose/cast/prearrange inputs per core. Returns list of 8 input dicts.

    Weight tensors are prearranged so that each DMA moves long contiguous
    per-partition runs:
      wkp/wvp[ki, ko, d]   = W.T[ko*128+ki, d]          (8KB runs)
      wqp[ki, g, ko, d]    = Wq.T[ko*128+ki, g*128+d]   (8KB runs per head)
      wop[oi, oo, h]       = Wo.T[oo*128+oi, h]         (2KB runs per slice)
    """
    x2 = np.ascontiguousarray(hidden_states.reshape(T, HID).T).astype(BF16NP)

    pos = positions.astype(np.float32)                      # [B, S]
    assert np.array_equal(positions[0], positions[1]), (
        "kernel assumes identical positions across batch"
    )
    half = HD // 2
    inv_freq = 1.0 / (ROPE_BASE ** (np.arange(half, dtype=np.float32) / half))
    ang = pos[0][:, None] * inv_freq[None, :]               # [S, 64]
    cosT = np.cos(ang).T                                    # [64, S]
    sinT = np.sin(ang).T
    cos2 = np.concatenate([cosT, cosT], axis=0).astype(BF16NP)    # [128, S]
    sin2 = np.concatenate([-sinT, sinT], axis=0).astype(BF16NP)

    r = np.arange(P)
    tri = (r[:, None] <= r[None, :]).astype(np.float32).astype(BF16NP)
    iden = np.eye(P, dtype=np.float32).astype(BF16NP)

    NKK = HID // P

    def prearrange_kd(wT):            # [HID, d] -> [ki, ko, d]
        d = wT.shape[1]
        return np.ascontiguousarray(
            wT.reshape(NKK, P, d).transpose(1, 0, 2)
        )

    in_maps = []
    for c in range(NCORES):
        qs = slice(c * HPC * HD, (c + 1) * HPC * HD)
        ks = slice(c * HD, (c + 1) * HD)
        wqT = Wq[qs, :].T.astype(BF16NP)                     # [HID, 512]
        wkT = Wk[ks, :].T.astype(BF16NP)                     # [HID, 128]
        wvT = Wv[ks, :].T.astype(BF16NP)
        woT = Wo[:, qs].T.astype(BF16NP)                     # [512, HID]
        wqp = np.ascontiguousarray(
            wqT.reshape(NKK, P, HPC, HD).transpose(1, 2, 0, 3)
        )                                                    # [ki, g, ko, d]
        wop = np.ascontiguousarray(
            woT.reshape(HPC, P, HID).transpose(1, 0, 2)
        )                                                    # [oi, oo, h]
        in_maps.append({
            "xT": x2,
            "wqp": wqp,
            "wkp": prearrange_kd(wkT),
            "wvp": prearrange_kd(wvT),
            "wop": wop,
            "cos2": cos2,
            "sin2": sin2,
            "tri": tri,
            "iden": iden,
        })
    return in_maps


def kernel(hidden_states, positions, Wq, Wk, Wv, Wo):
    from concourse.bass_utils import run_bass_kernel_spmd

    trace = bool(os.environ.get("CLAUDE_KERNEL_TRACE"))
    if trace:
        _install_ntff_hook()

    nc = _get_graph()
    in_maps = _host_prep(
        np.asarray(hidden_states), np.asarray(positions),
        np.asarray(Wq), np.asarray(Wk), np.asarray(Wv), np.asarray(Wo),
    )
    res = run_bass_kernel_spmd(
        nc, in_maps, core_ids=list(range(NCORES)), trace=trace,
    )
    LAST["exec_time_ns"] = res.exec_time_ns
    LAST["profile_json"] = res.profile_json
    if res.instructions_and_trace is not None:
        LAST["trace_path"] = res.instructions_and_trace[1]

    acc = np.zeros((T, HID), np.float32)
    for c in range(NCORES):
        acc += res.results[c]["out"].astype(np.float32)
    return acc.reshape(B, S, HID)
